# revision 5
# baseline (speedup 1.0000x reference)
"""Trainium2 Bass kernel for IR-Net style binarized 3x3 conv + BN + Hardtanh.

Reference computation:
  bw = sign(standardize(weight)) * sw   (sw = per-cout power-of-2 scale)
  ba = sign(x)
  y  = clip(conv3x3(ba, bw) * bn_scale + bn_bias, -1, 1)

Both matmul operands are exactly +-1, which is exactly representable in
fp8e4m3, so the conv runs as fp8 DoubleRow matmuls on the TensorEngine
with zero numerical error (fp32 PSUM accumulation of integers <= 2304).
All binarization is host-side prep: x ships as fp8 +-1 sign planes that
are already zero-padded and cin-chunk-interleaved, so activations DMA
straight into their SBUF matmul layout — no on-device binarize, border
memsets, or staging.  sw and the BN affine fold into one per-channel
scale/bias applied in the epilogue on VectorE.

Distribution: pure data parallel, 32 images -> 4 per NeuronCore, full
weights replicated, no collectives.

Layout: per-image zero-padded activation planes in SBUF, fp8, with the
two cin-128-chunks byte-interleaved as the DoubleRow k-subtile dim.
Rows are 57 wide (56 data + 1 shared zero column: col 0 is row r's left
pad AND row r-1's right pad), so each of the 9 conv taps is a contiguous
shifted window of the flattened plane and only 1 of every 57 output
columns is garbage.  The conv is 9 accumulated DoubleRow matmuls
([128,2,128] @ [128,2,456], K=256) per 8-row output tile.

The matmul stream runs at the DoubleRow issue-rate ceiling (1 column/
cycle, LDWEIGHTS fully pipelined), so the stream itself is at the fp8
peak; everything here is about the edges:

Startup: warmup matmuls read an UNWRITTEN scratch tile (garbage bits are
fine — the PSUM is never consumed), so they issue the moment TensorE
enters the kernel body and ramp the HAM clock gate with zero DMA/memset
dependency.  The first-needed data is split fine-grained across both
HWDGE rings in exact consumption order (img0 rows 0-10 halved across
rings, then per-tap weight pieces, then the next row bands), each piece
its own DMA so every matmul/LDWEIGHTS waits only on the bytes it
actually reads (Tile tracks sub-tile ranges and gives each DMA its own
completion semaphore).  The real stream then starts as soon as the first
block's data lands, running the first few matmuls at the still-throttled
clock — earlier than waiting for full warmup.  Later row bands, the
co=1 weights, and the bulk images go on the SWDGE ring (fast, but its
completion semaphores lag by a few us — fine for far-out consumers),
gated on early warmup/real matmuls so their HBM traffic doesn't contend
with the critical pieces.

Tail: output stores alternate between the two HWDGE rings, and the final
row-block of the last (img,co) is split into four 2-row PSUM groups
(small-N matmuls still issue at N+7 cycles — LDWEIGHTS never exposes) so
the last epilogue+store after the final matmul is ~1/4 size and the
post-stream drain the postamble waits on is short.
"""

import numpy as np

import concourse.bass as bass
import concourse.bacc as bacc
import concourse.mybir as mybir
import concourse.tile as tile
from concourse.bass_utils import run_bass_kernel_spmd

B, CIN, COUT, H, W = 32, 256, 256, 56, 56
NCORES = 8
BPC = B // NCORES            # images per core
HP = H + 2                   # padded rows
RW = W + 1                   # row width: 56 data + 1 shared zero col
IMG = HP * RW                # 3306
GUARD = 64                   # front zero guard (shifted windows stay in bounds)
XT = 3376                    # GUARD + IMG + tail guard(6); %16==0 for DoubleRow
RB = 8                       # output rows per tile
NBLK = H // RB               # 7
NCI = CIN // 128             # 2 cin chunks = DoubleRow k-subtiles
NCO = COUT // 128            # 2 cout chunks
KTAPS = 9
BN_EPS = 1e-5

# img0 band split points (tile elem index), in consumption order
SA = GUARD + 5 * RW          # imgA: rows 0-4   | imgB: rows 5-10
S1 = GUARD + 11 * RW         # rows 0-10 end
S2 = S1 + 8 * RW             # rows 11-18 end
S3 = S2 + 8 * RW             # rows 19-26 end
S4 = S3 + 16 * RW            # rows 27-42 end

NWARM = 6                    # warmup matmuls (throttled-clock PE busy bridge)
WN = 400                     # warmup matmul free dim (~333ns each throttled)

F32 = mybir.dt.float32
BF16 = mybir.dt.bfloat16
FP8 = mybir.dt.float8e4
FP8NP = mybir.dt.np(FP8)

_CACHE: dict = {}


def _build_nc() -> bass.Bass:
    nc = bacc.Bacc("TRN2", target_bir_lowering=False, debug=False, num_devices=NCORES)
    xin8 = nc.declare_dram_parameter("xin8", [BPC, 128, XT * NCI], FP8, isOutput=False)
    wts = nc.declare_dram_parameter(
        "wts", [128, KTAPS * NCO * NCI * 128], FP8, isOutput=False
    )
    sb = nc.declare_dram_parameter("sb", [128, 2 * NCO], F32, isOutput=False)
    yout = nc.declare_dram_parameter("yout", [BPC, COUT, H, W], BF16, isOutput=True)

    with tile.TileContext(nc) as tc:
        with (
            tc.tile_pool(name="const", bufs=1) as cpool,
            tc.tile_pool(name="psum", bufs=8, space=bass.MemorySpace.PSUM) as ppool,
            tc.tile_pool(name="ot", bufs=8) as otpool,
            tc.tile_pool(name="oc", bufs=12) as ocpool,
        ):
            # weights: [p, (co, k, j, m)]
            w_sb = cpool.tile([128, KTAPS * NCO * NCI * 128], FP8, tag="w")
            sb_sb = cpool.tile([128, 2 * NCO], F32, tag="sb")
            WTAP = NCI * 128          # 256 B per tap per partition
            WCO = KTAPS * WTAP        # one cout chunk
            w4 = w_sb.rearrange("p (co k j m) -> p k co j m", k=KTAPS, co=NCO, j=NCI)

            # Scratch for PE warmup operands.  Zeroed by two gpsimd memsets
            # (gpsimd reaches the kernel body first): the stationary 128
            # cols first so the first LDWEIGHTS waits only ~100ns of
            # memset, the moving remainder second.
            wz = cpool.tile([128, WN], FP8, tag="wz")
            nc.gpsimd.memset(wz[:, 0:128], 0.0)
            nc.gpsimd.memset(wz[:, 128:WN], 0.0)

            # Padded binarized activation planes, one tile per image;
            # entirely DMA-written (borders ship as zeros from the host).
            xp = {}
            for img in range(BPC):
                t = cpool.tile([128, XT, NCI], FP8, tag=f"xp{img}", name=f"xp{img}")
                xp[img] = t

            def ld_piece(img, a, b, eng):
                return eng.dma_start(
                    xp[img][:, a:b, :], xin8[img, :, a * NCI : b * NCI]
                )

            def ld_taps(k0, k1, eng, co=0):
                return eng.dma_start(
                    w_sb[:, co * WCO + k0 * WTAP : co * WCO + k1 * WTAP],
                    wts[:, co * WCO + k0 * WTAP : co * WCO + k1 * WTAP],
                )

            # PE warmup: dummy matmuls on the unwritten scratch tile (normal
            # fp8 mode, N=400 -> ~333ns each at the throttled clock).  They
            # have no dependencies at all, so PE busy starts at body entry
            # and the HAM clock gate (~3.4us of continuous busy) flips to
            # 8/8 shortly after the real stream begins.
            wm_ps = ppool.tile([128, WN], F32, tag="ps")
            wms = []
            for _ in range(NWARM):
                wms.append(nc.tensor.matmul(
                    wm_ps[:],
                    wz[:, 0:128],
                    wz[:, 0:WN],
                    start=True,
                    stop=True,
                ))

            # Startup DMAs, one piece per consumer group so each matmul /
            # LDWEIGHTS waits only on its own bytes (Tile gives every
            # dma_start its own completion semaphore and tracks sub-tile
            # byte ranges).  Ring order == issue order == consumption order.
            # ~0.7us of queue-engine time per descriptor is the pacing
            # constraint, so taps are paired after the first two.
            sq_chain = [
                ld_piece(0, 0, SA, nc.sync),          # img0 rows 0-4
                ld_taps(0, 1, nc.sync),               # tap 0
                ld_taps(2, 4, nc.sync),               # taps 2-3
                ld_taps(6, 8, nc.sync),               # taps 6-7
                ld_taps(8, 9, nc.sync),               # tap 8
                ld_piece(0, S2, S3, nc.sync),         # rows 19-26
            ]
            sc_chain = [
                ld_piece(0, SA, S1, nc.scalar),       # img0 rows 5-10
                ld_taps(1, 2, nc.scalar),             # tap 1
                ld_taps(4, 6, nc.scalar),             # taps 4-5
                ld_piece(0, S1, S2, nc.scalar),       # rows 11-18
                nc.scalar.dma_start(sb_sb[:], sb[:]),  # bn scale/bias
            ]
            gq_chain = []

            def gate_dma(dma, trigger):
                # real semaphore gate on an early trigger (so the transfer
                # starts promptly) plus a schedule-order-only edge after the
                # first real matmul (keeps the piece from being front-loaded
                # ahead of the critical startup set)
                tile.add_dep_helper(dma.ins, trigger.ins, sync=True,
                                    reason="JIT DMA trigger")
                tile.add_dep_helper(dma.ins, mm0.ins, sync=False,
                                    reason="keep behind critical startup")

            mm0 = None
            for img in range(BPC):
                for co in range(NCO):
                    if img == 0 and co == 1:
                        # co=1 weights + bulk images on the SWDGE ring: fast
                        # transfers but completion semaphores lag a few us —
                        # fine, their consumers are far out.  Gated on
                        # warmup / early-stream progress to stagger HBM
                        # traffic away from the critical startup pieces.
                        wc1 = ld_taps(0, KTAPS, nc.gpsimd, co=1)
                        gate_dma(wc1, wms[1])
                        gq_chain.append(wc1)
                        for im2, trig in ((1, wms[3]), (2, wms[5]),
                                          (3, mm0)):
                            dma = ld_piece(im2, 0, XT, nc.gpsimd)
                            gate_dma(dma, trig)
                            gq_chain.append(dma)
                    s_ap = sb_sb[:, co : co + 1]
                    b_ap = sb_sb[:, NCO + co : NCO + co + 1]
                    # (start padded row, rows, out queue) per output tile.
                    # outputs alternate between the two HWDGE rings (the
                    # gpsimd SWDGE ring's completions lag by several us,
                    # which would stretch the final drain the postamble
                    # waits on)
                    oqs = [nc.sync, nc.scalar]
                    blocks = [
                        (1 + b * RB, RB, oqs[b % 2]) for b in range(NBLK)
                    ]
                    if img == BPC - 1 and co == NCO - 1:
                        # final row-block as four 2-row PSUM groups: the
                        # epilogue+store chain after the very last matmul is
                        # ~1/4 size, and small-N matmuls still issue at N+7
                        # cycles (LDWEIGHTS never exposes), so this is free.
                        blocks = blocks[:-1] + [
                            (49, 2, nc.sync),
                            (51, 2, nc.scalar),
                            (53, 2, nc.sync),
                            (55, 2, nc.scalar),
                        ]
                    for bi, (y0p, rb, oq) in enumerate(blocks):
                        if img == 0 and co == 0 and bi == 1:
                            # img0 rows 27-57 on SWDGE (first consumed by
                            # block 3, >4us out so the laggy SWDGE sems are
                            # fine).  Must be EMITTED before the consuming
                            # blocks so Tile sees write-before-read.
                            for (a, b2), trig in (((S3, S4), wms[0]),
                                                  ((S4, XT), wms[0])):
                                dma = ld_piece(0, a, b2, nc.gpsimd)
                                gate_dma(dma, trig)
                                gq_chain.append(dma)
                        nt = rb * RW
                        ps = ppool.tile([128, nt], F32, tag="ps")
                        for k in range(KTAPS):
                            ky, kx = divmod(k, 3)
                            s0 = GUARD + (y0p + ky - 1) * RW + kx
                            rhs = xp[img][:, s0 : s0 + nt, :].rearrange(
                                "p x j -> p j x"
                            )
                            mm = nc.tensor.matmul(
                                ps[:],
                                w4[:, k, co],
                                rhs,
                                start=(k == 0),
                                stop=(k == KTAPS - 1),
                                perf_mode=mybir.MatmulPerfMode.DoubleRow,
                            )
                            if mm0 is None:
                                mm0 = mm
                        ot = otpool.tile([128, nt], F32, tag="ot")
                        nc.vector.tensor_scalar(
                            ot[:],
                            ps[:],
                            s_ap,
                            b_ap,
                            op0=mybir.AluOpType.mult,
                            op1=mybir.AluOpType.add,
                        )
                        # clip + compact away the garbage col per row, so
                        # both sides of the output DMA are fully contiguous.
                        # bf16 output: halves store traffic; quantization is
                        # ~2^-9 relative, far inside the accuracy budget.
                        oc = ocpool.tile([128, rb * W], BF16, tag="oc")
                        nc.vector.tensor_scalar(
                            oc[:],
                            ot.rearrange("p (r c) -> p r c", c=RW)[:, :, 0:W],
                            -1.0,
                            1.0,
                            op0=mybir.AluOpType.max,
                            op1=mybir.AluOpType.min,
                        )
                        # flat dest AP: rows of one channel are contiguous
                        # in DRAM, so this coalesces each partition's store
                        # into one rb*56-elem run instead of per-row pieces
                        oq.dma_start(
                            yout[
                                img, co * 128 : (co + 1) * 128, y0p - 1 : y0p - 1 + rb, :
                            ].rearrange("p r x -> p (r x)"),
                            oc[:],
                        )
            # pin issue order per ring (ring packet order = issue order)
            for ch in (sc_chain, sq_chain, gq_chain):
                for a, b in zip(ch, ch[1:]):
                    tile.add_dep_helper(
                        b.ins, a.ins, sync=False, reason="startup DMA issue order"
                    )
    nc.finalize()
    return nc


def get_nc() -> bass.Bass:
    if "nc" not in _CACHE:
        _CACHE["nc"] = _build_nc()
    return _CACHE["nc"]


def _host_prep(weight, gamma, beta, running_mean, running_var):
    """Binarize standardized weights, fold sw + BN into scale/bias."""
    wf = weight.reshape(COUT, -1).astype(np.float64)
    n = wf.shape[1]
    mean = wf.mean(axis=1, keepdims=True)
    d = wf - mean
    sgn = np.where(d >= 0, 1.0, -1.0)
    std = np.sqrt((d * d).sum(axis=1, keepdims=True) / (n - 1))
    bw = d / std
    sw = np.exp2(np.round(np.log2(np.abs(bw).mean(axis=1))))  # [COUT]
    inv = gamma.astype(np.float64) / np.sqrt(running_var.astype(np.float64) + BN_EPS)
    scale = (sw * inv).astype(np.float32)
    bias = (beta.astype(np.float64) - running_mean.astype(np.float64) * inv).astype(
        np.float32
    )

    # wts[p, (co, k, j, m)] = sgn[co*128+m, (j*128+p)*9 + k]
    w6 = sgn.reshape(NCO, 128, NCI, 128, KTAPS)  # [co, m, j, p, k]
    wts = (
        np.ascontiguousarray(np.transpose(w6, (3, 0, 4, 2, 1)))  # p co k j m
        .reshape(128, KTAPS * NCO * NCI * 128)
        .astype(FP8NP)
    )
    # sb[m, co] = scale chunk, sb[m, NCO+co] = bias chunk
    sbarr = np.concatenate(
        [scale.reshape(NCO, 128).T, bias.reshape(NCO, 128).T], axis=1
    ).astype(np.float32)
    sbarr = np.ascontiguousarray(sbarr)
    return wts, sbarr


def _host_signs(x):
    """fp8 +-1 sign planes, zero-padded 58x57 rows, cin-chunk interleaved.

    out[b, p, t, j] = fp8(sign(x[b, j*128+p, r-1, c-1])) at t = GUARD+r*57+c
    for the interior, 0 elsewhere (pads/guards), matching torch.sign
    (sign(0) = 0).
    """
    xv = x.reshape(B, NCI, 128, H, W)
    xs = ((xv < 0).astype(np.uint8) * 0x80) | ((xv != 0).astype(np.uint8) * 0x38)
    out = np.zeros((B, 128, XT, NCI), np.uint8)
    interior = out[:, :, GUARD : GUARD + IMG, :].reshape(B, 128, HP, RW, NCI)
    interior[:, :, 1 : H + 1, 1 : W + 1, :] = xs.transpose(0, 2, 3, 4, 1)
    return out.reshape(B, 128, XT * NCI).view(FP8NP)


def run(x, weight, gamma, beta, running_mean, running_var, trace=False, **tkw):
    x = np.asarray(x, dtype=np.float32)
    wts, sbarr = _host_prep(
        np.asarray(weight, dtype=np.float32),
        np.asarray(gamma, dtype=np.float32),
        np.asarray(beta, dtype=np.float32),
        np.asarray(running_mean, dtype=np.float32),
        np.asarray(running_var, dtype=np.float32),
    )
    x8 = _host_signs(x)
    in_maps = [
        {
            "xin8": x8[c * BPC : (c + 1) * BPC],
            "wts": wts,
            "sb": sbarr,
        }
        for c in range(NCORES)
    ]
    nc = get_nc()
    res = run_bass_kernel_spmd(nc, in_maps, list(range(NCORES)), trace=trace, **tkw)
    y = np.concatenate([r["yout"] for r in res.results], axis=0)
    return y.astype(np.float32, copy=False), res


def kernel(x, weight, gamma, beta, running_mean, running_var):
    y, _ = run(x, weight, gamma, beta, running_mean, running_var)
    return y


# revision 10
# speedup vs baseline: 1.0211x; 1.0211x over previous
"""Trainium2 Bass kernel for IR-Net style binarized 3x3 conv + BN + Hardtanh.

Reference computation:
  bw = sign(standardize(weight)) * sw   (sw = per-cout power-of-2 scale)
  ba = sign(x)
  y  = clip(conv3x3(ba, bw) * bn_scale + bn_bias, -1, 1)

Both matmul operands are exactly +-1, which is exactly representable in
fp8e4m3, so the conv runs as fp8 DoubleRow matmuls on the TensorEngine
with zero numerical error (fp32 PSUM accumulation of integers <= 2304).
All binarization is host-side prep: x ships as fp8 +-1 sign planes that
are already zero-padded and cin-chunk-interleaved, so activations DMA
straight into their SBUF matmul layout — no on-device binarize, border
memsets, or staging.  sw and the BN affine fold into one per-channel
scale/bias applied in the epilogue on VectorE.

Distribution: pure data parallel, 32 images -> 4 per NeuronCore, full
weights replicated, no collectives.

Layout: per-image zero-padded activation planes in SBUF, fp8, with the
two cin-128-chunks byte-interleaved as the DoubleRow k-subtile dim.
Rows are 57 wide (56 data + 1 shared zero column: col 0 is row r's left
pad AND row r-1's right pad), so each of the 9 conv taps is a contiguous
shifted window of the flattened plane and only 1 of every 57 output
columns is garbage.  The conv is 9 accumulated DoubleRow matmuls
([128,2,128] @ [128,2,456], K=256) per 8-row output tile.

The matmul stream runs at the DoubleRow issue-rate ceiling (1 column/
cycle, LDWEIGHTS fully pipelined), so the stream itself is at the fp8
peak; everything here is about the edges:

Startup: PE busy begins at body entry (~7.7us) with warmup matmuls on a
zeroed scratch tile; the HAM clock gate un-throttles after ~5.4us of
UNINTERRUPTED busy (~13.1us — any gap or data stall resets the ramp).
DMA completion carries ~2.2us fixed latency plus large per-DMA
overheads, so the critical set ships as few LARGE pieces: weight taps
0-6 lead the scalar ring, img0 rows 0-10 lead the sync ring — both land
~10.7us.  Warmup is sized to hand off gaplessly to the real stream at
~11.4us, whose first ~5 matmuls run at the throttled clock until the
gate flips — banking ~1us versus warming all the way to the flip.
Later row bands, the co=1 weights, and the bulk images go on the SWDGE
ring (fast, but its completion semaphores lag by a few us — fine for
far-out consumers), gated on warmup progress so their HBM traffic
doesn't contend with the critical pieces.

Tail: output stores alternate between the two HWDGE rings, and the final
row-block of the last (img,co) is split into four 2-row PSUM groups
(small-N matmuls still issue at N+7 cycles — LDWEIGHTS never exposes) so
the last epilogue+store after the final matmul is ~1/4 size and the
post-stream drain the postamble waits on is short.
"""

import numpy as np

import concourse.bass as bass
import concourse.bacc as bacc
import concourse.mybir as mybir
import concourse.tile as tile
from concourse.bass_utils import run_bass_kernel_spmd

B, CIN, COUT, H, W = 32, 256, 256, 56, 56
NCORES = 8
BPC = B // NCORES            # images per core
HP = H + 2                   # padded rows
RW = W + 1                   # row width: 56 data + 1 shared zero col
IMG = HP * RW                # 3306
GUARD = 64                   # front zero guard (shifted windows stay in bounds)
XT = 3376                    # GUARD + IMG + tail guard(6); %16==0 for DoubleRow
RB = 8                       # output rows per tile
NBLK = H // RB               # 7
NCI = CIN // 128             # 2 cin chunks = DoubleRow k-subtiles
NCO = COUT // 128            # 2 cout chunks
KTAPS = 9
BN_EPS = 1e-5

# img0 band split points (tile elem index)
S1 = GUARD + 11 * RW         # rows 0-10 end (first block's reach)
S3 = S1 + 16 * RW            # rows 11-26 end

NWARM = 11                   # warmup matmuls (throttled-clock PE busy bridge)
WN = 400                     # warmup matmul free dim (~333ns each throttled)

F32 = mybir.dt.float32
BF16 = mybir.dt.bfloat16
FP8 = mybir.dt.float8e4
FP8NP = mybir.dt.np(FP8)

_CACHE: dict = {}


def _build_nc() -> bass.Bass:
    nc = bacc.Bacc("TRN2", target_bir_lowering=False, debug=False, num_devices=NCORES)
    xin8 = nc.declare_dram_parameter("xin8", [BPC, 128, XT * NCI], FP8, isOutput=False)
    wts = nc.declare_dram_parameter(
        "wts", [128, KTAPS * NCO * NCI * 128], FP8, isOutput=False
    )
    sb = nc.declare_dram_parameter("sb", [128, 2 * NCO], F32, isOutput=False)
    yout = nc.declare_dram_parameter("yout", [BPC, COUT, H, W], BF16, isOutput=True)

    with tile.TileContext(nc) as tc:
        with (
            tc.tile_pool(name="const", bufs=1) as cpool,
            tc.tile_pool(name="psum", bufs=8, space=bass.MemorySpace.PSUM) as ppool,
            tc.tile_pool(name="ot", bufs=8) as otpool,
            tc.tile_pool(name="oc", bufs=12) as ocpool,
        ):
            # weights: [p, (co, k, j, m)]
            w_sb = cpool.tile([128, KTAPS * NCO * NCI * 128], FP8, tag="w")
            sb_sb = cpool.tile([128, 2 * NCO], F32, tag="sb")
            WTAP = NCI * 128          # 256 B per tap per partition
            WCO = KTAPS * WTAP        # one cout chunk
            w4 = w_sb.rearrange("p (co k j m) -> p k co j m", k=KTAPS, co=NCO, j=NCI)

            # Scratch for PE warmup operands.  Zeroed by two gpsimd memsets
            # (gpsimd reaches the kernel body first): the stationary 128
            # cols first so the first LDWEIGHTS waits only ~100ns of
            # memset, the moving remainder second.
            wz = cpool.tile([128, WN], FP8, tag="wz")
            nc.gpsimd.memset(wz[:, 0:128], 0.0)
            nc.gpsimd.memset(wz[:, 128:WN], 0.0)

            # Padded binarized activation planes, one tile per image;
            # entirely DMA-written (borders ship as zeros from the host).
            xp = {}
            for img in range(BPC):
                t = cpool.tile([128, XT, NCI], FP8, tag=f"xp{img}", name=f"xp{img}")
                xp[img] = t

            def ld_piece(img, a, b, eng):
                return eng.dma_start(
                    xp[img][:, a:b, :], xin8[img, :, a * NCI : b * NCI]
                )

            def ld_taps(k0, k1, eng, co=0):
                return eng.dma_start(
                    w_sb[:, co * WCO + k0 * WTAP : co * WCO + k1 * WTAP],
                    wts[:, co * WCO + k0 * WTAP : co * WCO + k1 * WTAP],
                )

            # PE warmup: dummy matmuls on the zeroed scratch tile (normal
            # fp8 mode, N=400 -> ~333ns each at the throttled clock).  PE
            # busy starts at body entry; the HAM un-throttle fires ~5.4us
            # of continuous busy later (~13.1us).  Warmup is sized so the
            # real stream starts gaplessly at ~11.4us — data is already
            # there — running its first ~5 matmuls at the throttled clock
            # before the gate flips, which banks ~1us versus warming all
            # the way to the flip.  Any PE idle gap here delays the
            # un-throttle, so the handoff must stay seamless.
            wm_ps = ppool.tile([128, WN], F32, tag="ps")
            wms = []
            for _ in range(NWARM):
                wms.append(nc.tensor.matmul(
                    wm_ps[:],
                    wz[:, 0:128],
                    wz[:, 0:WN],
                    start=True,
                    stop=True,
                ))

            # Startup DMAs.  DMA completion has ~2.2us fixed latency and
            # large per-DMA overhead (descriptor ~0.7us of queue-engine
            # time, ring service per piece), so the critical set ships as
            # FEW, LARGE pieces: weights taps 0-6 lead the scalar ring
            # (first LDWEIGHTS gate), img0 rows 0-10 lead the sync ring
            # (first matmul gate); both land ~10.7us < stream start.
            sq_chain = [
                ld_piece(0, 0, S1, nc.sync),          # img0 rows 0-10
                ld_taps(7, 9, nc.sync),               # taps 7-8
            ]
            sc_chain = [
                ld_taps(0, 7, nc.scalar),             # taps 0-6
                nc.scalar.dma_start(sb_sb[:], sb[:]),  # bn scale/bias
            ]
            gq_chain = []

            def gate_dma(dma, trigger):
                # real semaphore gate on an early trigger (so the transfer
                # starts promptly) plus a schedule-order-only edge after the
                # first real matmul (keeps the piece from being front-loaded
                # ahead of the critical startup set)
                tile.add_dep_helper(dma.ins, trigger.ins, sync=True,
                                    reason="JIT DMA trigger")
                tile.add_dep_helper(dma.ins, mm0.ins, sync=False,
                                    reason="keep behind critical startup")

            mm0 = None
            for img in range(BPC):
                for co in range(NCO):
                    if img == 0 and co == 1:
                        # co=1 weights + bulk images on the SWDGE ring: fast
                        # transfers but completion semaphores lag a few us —
                        # fine, their consumers are far out.  Gated on
                        # warmup / early-stream progress to stagger HBM
                        # traffic away from the critical startup pieces.
                        wc1 = ld_taps(0, KTAPS, nc.gpsimd, co=1)
                        gate_dma(wc1, wms[2])
                        gq_chain.append(wc1)
                        for im2, trig in ((1, wms[4]), (2, wms[6]),
                                          (3, mm0)):
                            dma = ld_piece(im2, 0, XT, nc.gpsimd)
                            gate_dma(dma, trig)
                            gq_chain.append(dma)
                    s_ap = sb_sb[:, co : co + 1]
                    b_ap = sb_sb[:, NCO + co : NCO + co + 1]
                    # (start padded row, rows, out queue) per output tile.
                    # outputs alternate between the two HWDGE rings (the
                    # gpsimd SWDGE ring's completions lag by several us,
                    # which would stretch the final drain the postamble
                    # waits on)
                    oqs = [nc.sync, nc.scalar]
                    blocks = [
                        (1 + b * RB, RB, oqs[b % 2]) for b in range(NBLK)
                    ]
                    if img == BPC - 1 and co == NCO - 1:
                        # final row-block as four 2-row PSUM groups: the
                        # epilogue+store chain after the very last matmul is
                        # ~1/4 size, and small-N matmuls still issue at N+7
                        # cycles (LDWEIGHTS never exposes), so this is free.
                        blocks = blocks[:-1] + [
                            (49, 2, nc.sync),
                            (51, 2, nc.scalar),
                            (53, 2, nc.sync),
                            (55, 2, nc.scalar),
                        ]
                    for bi, (y0p, rb, oq) in enumerate(blocks):
                        if img == 0 and co == 0 and bi == 1:
                            # img0 rows 11-57 on SWDGE (fast transfers,
                            # laggy completion sems — consumers are >2us
                            # out).  Must be EMITTED before the consuming
                            # blocks so Tile sees write-before-read.
                            for (a, b2), trig in (((S1, S3), wms[0]),
                                                  ((S3, XT), wms[1])):
                                dma = ld_piece(0, a, b2, nc.gpsimd)
                                gate_dma(dma, trig)
                                gq_chain.append(dma)
                        nt = rb * RW
                        ps = ppool.tile([128, nt], F32, tag="ps")
                        for k in range(KTAPS):
                            ky, kx = divmod(k, 3)
                            s0 = GUARD + (y0p + ky - 1) * RW + kx
                            rhs = xp[img][:, s0 : s0 + nt, :].rearrange(
                                "p x j -> p j x"
                            )
                            mm = nc.tensor.matmul(
                                ps[:],
                                w4[:, k, co],
                                rhs,
                                start=(k == 0),
                                stop=(k == KTAPS - 1),
                                perf_mode=mybir.MatmulPerfMode.DoubleRow,
                            )
                            if mm0 is None:
                                mm0 = mm
                        ot = otpool.tile([128, nt], F32, tag="ot")
                        nc.vector.tensor_scalar(
                            ot[:],
                            ps[:],
                            s_ap,
                            b_ap,
                            op0=mybir.AluOpType.mult,
                            op1=mybir.AluOpType.add,
                        )
                        # clip + compact away the garbage col per row, so
                        # both sides of the output DMA are fully contiguous.
                        # bf16 output: halves store traffic; quantization is
                        # ~2^-9 relative, far inside the accuracy budget.
                        oc = ocpool.tile([128, rb * W], BF16, tag="oc")
                        nc.vector.tensor_scalar(
                            oc[:],
                            ot.rearrange("p (r c) -> p r c", c=RW)[:, :, 0:W],
                            -1.0,
                            1.0,
                            op0=mybir.AluOpType.max,
                            op1=mybir.AluOpType.min,
                        )
                        # flat dest AP: rows of one channel are contiguous
                        # in DRAM, so this coalesces each partition's store
                        # into one rb*56-elem run instead of per-row pieces
                        oq.dma_start(
                            yout[
                                img, co * 128 : (co + 1) * 128, y0p - 1 : y0p - 1 + rb, :
                            ].rearrange("p r x -> p (r x)"),
                            oc[:],
                        )
            # pin issue order per ring (ring packet order = issue order)
            for ch in (sc_chain, sq_chain, gq_chain):
                for a, b in zip(ch, ch[1:]):
                    tile.add_dep_helper(
                        b.ins, a.ins, sync=False, reason="startup DMA issue order"
                    )
    nc.finalize()
    return nc


def get_nc() -> bass.Bass:
    if "nc" not in _CACHE:
        _CACHE["nc"] = _build_nc()
    return _CACHE["nc"]


def _host_prep(weight, gamma, beta, running_mean, running_var):
    """Binarize standardized weights, fold sw + BN into scale/bias."""
    wf = weight.reshape(COUT, -1).astype(np.float64)
    n = wf.shape[1]
    mean = wf.mean(axis=1, keepdims=True)
    d = wf - mean
    sgn = np.where(d >= 0, 1.0, -1.0)
    std = np.sqrt((d * d).sum(axis=1, keepdims=True) / (n - 1))
    bw = d / std
    sw = np.exp2(np.round(np.log2(np.abs(bw).mean(axis=1))))  # [COUT]
    inv = gamma.astype(np.float64) / np.sqrt(running_var.astype(np.float64) + BN_EPS)
    scale = (sw * inv).astype(np.float32)
    bias = (beta.astype(np.float64) - running_mean.astype(np.float64) * inv).astype(
        np.float32
    )

    # wts[p, (co, k, j, m)] = sgn[co*128+m, (j*128+p)*9 + k]
    w6 = sgn.reshape(NCO, 128, NCI, 128, KTAPS)  # [co, m, j, p, k]
    wts = (
        np.ascontiguousarray(np.transpose(w6, (3, 0, 4, 2, 1)))  # p co k j m
        .reshape(128, KTAPS * NCO * NCI * 128)
        .astype(FP8NP)
    )
    # sb[m, co] = scale chunk, sb[m, NCO+co] = bias chunk
    sbarr = np.concatenate(
        [scale.reshape(NCO, 128).T, bias.reshape(NCO, 128).T], axis=1
    ).astype(np.float32)
    sbarr = np.ascontiguousarray(sbarr)
    return wts, sbarr


def _host_signs(x):
    """fp8 +-1 sign planes, zero-padded 58x57 rows, cin-chunk interleaved.

    out[b, p, t, j] = fp8(sign(x[b, j*128+p, r-1, c-1])) at t = GUARD+r*57+c
    for the interior, 0 elsewhere (pads/guards), matching torch.sign
    (sign(0) = 0).
    """
    xv = x.reshape(B, NCI, 128, H, W)
    xs = ((xv < 0).astype(np.uint8) * 0x80) | ((xv != 0).astype(np.uint8) * 0x38)
    out = np.zeros((B, 128, XT, NCI), np.uint8)
    interior = out[:, :, GUARD : GUARD + IMG, :].reshape(B, 128, HP, RW, NCI)
    interior[:, :, 1 : H + 1, 1 : W + 1, :] = xs.transpose(0, 2, 3, 4, 1)
    return out.reshape(B, 128, XT * NCI).view(FP8NP)


def run(x, weight, gamma, beta, running_mean, running_var, trace=False, **tkw):
    x = np.asarray(x, dtype=np.float32)
    wts, sbarr = _host_prep(
        np.asarray(weight, dtype=np.float32),
        np.asarray(gamma, dtype=np.float32),
        np.asarray(beta, dtype=np.float32),
        np.asarray(running_mean, dtype=np.float32),
        np.asarray(running_var, dtype=np.float32),
    )
    x8 = _host_signs(x)
    in_maps = [
        {
            "xin8": x8[c * BPC : (c + 1) * BPC],
            "wts": wts,
            "sb": sbarr,
        }
        for c in range(NCORES)
    ]
    nc = get_nc()
    res = run_bass_kernel_spmd(nc, in_maps, list(range(NCORES)), trace=trace, **tkw)
    y = np.concatenate([r["yout"] for r in res.results], axis=0)
    return y.astype(np.float32, copy=False), res


def kernel(x, weight, gamma, beta, running_mean, running_var):
    y, _ = run(x, weight, gamma, beta, running_mean, running_var)
    return y


# revision 12
# speedup vs baseline: 1.0317x; 1.0104x over previous
"""Trainium2 Bass kernel for IR-Net style binarized 3x3 conv + BN + Hardtanh.

Reference computation:
  bw = sign(standardize(weight)) * sw   (sw = per-cout power-of-2 scale)
  ba = sign(x)
  y  = clip(conv3x3(ba, bw) * bn_scale + bn_bias, -1, 1)

Both matmul operands are exactly +-1, which is exactly representable in
fp8e4m3, so the conv runs as fp8 DoubleRow matmuls on the TensorEngine
with zero numerical error (fp32 PSUM accumulation of integers <= 2304).
All binarization is host-side prep: x ships as fp8 +-1 sign planes that
are already zero-padded and cin-chunk-interleaved, so activations DMA
straight into their SBUF matmul layout — no on-device binarize, border
memsets, or staging.  sw and the BN affine fold into one per-channel
scale/bias applied in the epilogue on VectorE.

Distribution: pure data parallel, 32 images -> 4 per NeuronCore, full
weights replicated, no collectives.

Layout: per-image zero-padded activation planes in SBUF, fp8, with the
two cin-128-chunks byte-interleaved as the DoubleRow k-subtile dim.
Rows are 57 wide (56 data + 1 shared zero column: col 0 is row r's left
pad AND row r-1's right pad), so each of the 9 conv taps is a contiguous
shifted window of the flattened plane and only 1 of every 57 output
columns is garbage.  The conv is 9 accumulated DoubleRow matmuls
([128,2,128] @ [128,2,456], K=256) per 8-row output tile.

The matmul stream runs at the DoubleRow issue-rate ceiling (1 column/
cycle, LDWEIGHTS fully pipelined), so the stream itself is at the fp8
peak; everything here is about the edges:

Startup: PE busy begins at body entry (~7.7us) with warmup matmuls on a
zeroed scratch tile; the HAM clock gate un-throttles after ~5.4us of
UNINTERRUPTED busy (~13.1us — any gap or data stall resets the ramp).
DMA completion carries ~2.2us fixed latency plus large per-DMA
overheads, so the critical set ships as few LARGE pieces: weight taps
0-6 lead the scalar ring, img0 rows 0-10 lead the sync ring — both land
~10.7us.  Warmup is sized to hand off gaplessly to the real stream at
~11.4us, whose first ~5 matmuls run at the throttled clock until the
gate flips — banking ~1us versus warming all the way to the flip.
Later row bands, the co=1 weights, and the bulk images go on the SWDGE
ring (fast, but its completion semaphores lag by a few us — fine for
far-out consumers), gated on warmup progress so their HBM traffic
doesn't contend with the critical pieces.

Tail: output stores alternate between the two HWDGE rings, and the final
row-block of the last (img,co) is split into four 2-row PSUM groups
(small-N matmuls still issue at N+7 cycles — LDWEIGHTS never exposes) so
the last epilogue+store after the final matmul is ~1/4 size and the
post-stream drain the postamble waits on is short.
"""

import numpy as np

import concourse.bass as bass
import concourse.bacc as bacc
import concourse.mybir as mybir
import concourse.tile as tile
from concourse.bass_utils import run_bass_kernel_spmd

B, CIN, COUT, H, W = 32, 256, 256, 56, 56
NCORES = 8
BPC = B // NCORES            # images per core
HP = H + 2                   # padded rows
RW = W + 1                   # row width: 56 data + 1 shared zero col
IMG = HP * RW                # 3306
GUARD = 64                   # front zero guard (shifted windows stay in bounds)
XT = 3376                    # GUARD + IMG + tail guard(6); %16==0 for DoubleRow
RB = 8                       # output rows per tile
NBLK = H // RB               # 7
NCI = CIN // 128             # 2 cin chunks = DoubleRow k-subtiles
NCO = COUT // 128            # 2 cout chunks
KTAPS = 9
BN_EPS = 1e-5

# img0 band split points (tile elem index)
S1 = GUARD + 11 * RW         # rows 0-10 end (first block's reach)
S3 = S1 + 16 * RW            # rows 11-26 end

NWARM = 10                   # warmup matmuls (throttled-clock PE busy bridge)
WN = 400                     # warmup matmul free dim (~333ns each throttled)

F32 = mybir.dt.float32
BF16 = mybir.dt.bfloat16
FP8 = mybir.dt.float8e4
FP8NP = mybir.dt.np(FP8)

_CACHE: dict = {}


def _build_nc() -> bass.Bass:
    nc = bacc.Bacc("TRN2", target_bir_lowering=False, debug=False, num_devices=NCORES)
    xin8 = nc.declare_dram_parameter("xin8", [BPC, 128, XT * NCI], FP8, isOutput=False)
    wts = nc.declare_dram_parameter(
        "wts", [128, KTAPS * NCO * NCI * 128], FP8, isOutput=False
    )
    sb = nc.declare_dram_parameter("sb", [128, 2 * NCO], F32, isOutput=False)
    yout = nc.declare_dram_parameter("yout", [BPC, COUT, H, W], BF16, isOutput=True)

    with tile.TileContext(nc) as tc:
        with (
            tc.tile_pool(name="const", bufs=1) as cpool,
            tc.tile_pool(name="psum", bufs=8, space=bass.MemorySpace.PSUM) as ppool,
            tc.tile_pool(name="ot", bufs=8) as otpool,
            tc.tile_pool(name="oc", bufs=12) as ocpool,
        ):
            # weights: [p, (co, k, j, m)]
            w_sb = cpool.tile([128, KTAPS * NCO * NCI * 128], FP8, tag="w")
            sb_sb = cpool.tile([128, 2 * NCO], F32, tag="sb")
            WTAP = NCI * 128          # 256 B per tap per partition
            WCO = KTAPS * WTAP        # one cout chunk
            w4 = w_sb.rearrange("p (co k j m) -> p k co j m", k=KTAPS, co=NCO, j=NCI)

            # Scratch for PE warmup operands.  Zeroed by two gpsimd memsets
            # (gpsimd reaches the kernel body first): the stationary 128
            # cols first so the first LDWEIGHTS waits only ~100ns of
            # memset, the moving remainder second.
            wz = cpool.tile([128, WN], FP8, tag="wz")
            nc.gpsimd.memset(wz[:, 0:128], 0.0)
            nc.gpsimd.memset(wz[:, 128:WN], 0.0)

            # Padded binarized activation planes, one tile per image;
            # entirely DMA-written (borders ship as zeros from the host).
            xp = {}
            for img in range(BPC):
                t = cpool.tile([128, XT, NCI], FP8, tag=f"xp{img}", name=f"xp{img}")
                xp[img] = t

            def ld_piece(img, a, b, eng):
                return eng.dma_start(
                    xp[img][:, a:b, :], xin8[img, :, a * NCI : b * NCI]
                )

            def ld_taps(k0, k1, eng, co=0):
                return eng.dma_start(
                    w_sb[:, co * WCO + k0 * WTAP : co * WCO + k1 * WTAP],
                    wts[:, co * WCO + k0 * WTAP : co * WCO + k1 * WTAP],
                )

            # PE warmup: dummy matmuls on the zeroed scratch tile (normal
            # fp8 mode, N=400 -> ~333ns each at the throttled clock).  PE
            # busy starts at body entry; the HAM un-throttle fires ~5.4us
            # of continuous busy later (~13.1us).  Warmup is sized so the
            # real stream starts gaplessly at ~11.4us — data is already
            # there — running its first ~5 matmuls at the throttled clock
            # before the gate flips, which banks ~1us versus warming all
            # the way to the flip.  Any PE idle gap here delays the
            # un-throttle, so the handoff must stay seamless.
            wm_ps = ppool.tile([128, WN], F32, tag="ps")
            wms = []
            for _ in range(NWARM):
                wms.append(nc.tensor.matmul(
                    wm_ps[:],
                    wz[:, 0:128],
                    wz[:, 0:WN],
                    start=True,
                    stop=True,
                ))

            # Startup DMAs.  DMA completion has ~2.2us fixed latency and
            # large per-DMA overhead (descriptor ~0.7us of queue-engine
            # time, ring service per piece), so the critical set ships as
            # FEW, LARGE pieces: weights taps 0-6 lead the scalar ring
            # (first LDWEIGHTS gate), img0 rows 0-10 lead the sync ring
            # (first matmul gate); both land ~10.7us < stream start.
            sq_chain = [
                ld_piece(0, 0, S1, nc.sync),          # img0 rows 0-10
                ld_taps(7, 9, nc.sync),               # taps 7-8
            ]
            sc_chain = [
                ld_taps(0, 7, nc.scalar),             # taps 0-6
                nc.scalar.dma_start(sb_sb[:], sb[:]),  # bn scale/bias
            ]
            gq_chain = []

            def gate_dma(dma, trigger):
                # real semaphore gate on an early trigger (so the transfer
                # starts promptly) plus a schedule-order-only edge after the
                # first real matmul (keeps the piece from being front-loaded
                # ahead of the critical startup set)
                tile.add_dep_helper(dma.ins, trigger.ins, sync=True,
                                    reason="JIT DMA trigger")
                tile.add_dep_helper(dma.ins, mm0.ins, sync=False,
                                    reason="keep behind critical startup")

            mm0 = None
            for img in range(BPC):
                for co in range(NCO):
                    if img == 0 and co == 1:
                        # co=1 weights + bulk images on the SWDGE ring: fast
                        # transfers but completion semaphores lag a few us —
                        # fine, their consumers are far out.  Gated on
                        # warmup / early-stream progress to stagger HBM
                        # traffic away from the critical startup pieces.
                        wc1 = ld_taps(0, KTAPS, nc.gpsimd, co=1)
                        gate_dma(wc1, wms[2])
                        gq_chain.append(wc1)
                        for im2, trig in ((1, wms[4]), (2, wms[6]),
                                          (3, mm0)):
                            dma = ld_piece(im2, 0, XT, nc.gpsimd)
                            gate_dma(dma, trig)
                            gq_chain.append(dma)
                    s_ap = sb_sb[:, co : co + 1]
                    b_ap = sb_sb[:, NCO + co : NCO + co + 1]
                    # (start padded row, rows, out queue) per output tile.
                    # outputs alternate between the two HWDGE rings (the
                    # gpsimd SWDGE ring's completions lag by several us,
                    # which would stretch the final drain the postamble
                    # waits on)
                    oqs = [nc.sync, nc.scalar]
                    blocks = [
                        (1 + b * RB, RB, oqs[b % 2]) for b in range(NBLK)
                    ]
                    if img == BPC - 1 and co == NCO - 1:
                        # final row-block split 4+4 across both rings: the
                        # epilogue+store after the very last matmul halves,
                        # and the two drains overlap.  No finer — each
                        # store costs ~1.6-2us of ring-packet time however
                        # small, and the drain must finish inside the
                        # ~6.4us semaphore-reset postamble that runs
                        # concurrently after the last engine instruction.
                        blocks = blocks[:-1] + [
                            (49, 4, nc.sync),
                            (53, 4, nc.scalar),
                        ]
                    for bi, (y0p, rb, oq) in enumerate(blocks):
                        if img == 0 and co == 0 and bi == 1:
                            # img0 rows 11-57 on SWDGE (fast transfers,
                            # laggy completion sems — consumers are >2us
                            # out).  Must be EMITTED before the consuming
                            # blocks so Tile sees write-before-read.
                            for (a, b2), trig in (((S1, S3), wms[0]),
                                                  ((S3, XT), wms[1])):
                                dma = ld_piece(0, a, b2, nc.gpsimd)
                                gate_dma(dma, trig)
                                gq_chain.append(dma)
                        nt = rb * RW
                        ps = ppool.tile([128, nt], F32, tag="ps")
                        for k in range(KTAPS):
                            ky, kx = divmod(k, 3)
                            s0 = GUARD + (y0p + ky - 1) * RW + kx
                            rhs = xp[img][:, s0 : s0 + nt, :].rearrange(
                                "p x j -> p j x"
                            )
                            mm = nc.tensor.matmul(
                                ps[:],
                                w4[:, k, co],
                                rhs,
                                start=(k == 0),
                                stop=(k == KTAPS - 1),
                                perf_mode=mybir.MatmulPerfMode.DoubleRow,
                            )
                            if mm0 is None:
                                mm0 = mm
                        ot = otpool.tile([128, nt], F32, tag="ot")
                        nc.vector.tensor_scalar(
                            ot[:],
                            ps[:],
                            s_ap,
                            b_ap,
                            op0=mybir.AluOpType.mult,
                            op1=mybir.AluOpType.add,
                        )
                        # clip + compact away the garbage col per row, so
                        # both sides of the output DMA are fully contiguous.
                        # bf16 output: halves store traffic; quantization is
                        # ~2^-9 relative, far inside the accuracy budget.
                        oc = ocpool.tile([128, rb * W], BF16, tag="oc")
                        nc.vector.tensor_scalar(
                            oc[:],
                            ot.rearrange("p (r c) -> p r c", c=RW)[:, :, 0:W],
                            -1.0,
                            1.0,
                            op0=mybir.AluOpType.max,
                            op1=mybir.AluOpType.min,
                        )
                        # flat dest AP: rows of one channel are contiguous
                        # in DRAM, so this coalesces each partition's store
                        # into one rb*56-elem run instead of per-row pieces
                        oq.dma_start(
                            yout[
                                img, co * 128 : (co + 1) * 128, y0p - 1 : y0p - 1 + rb, :
                            ].rearrange("p r x -> p (r x)"),
                            oc[:],
                        )
            # pin issue order per ring (ring packet order = issue order)
            for ch in (sc_chain, sq_chain, gq_chain):
                for a, b in zip(ch, ch[1:]):
                    tile.add_dep_helper(
                        b.ins, a.ins, sync=False, reason="startup DMA issue order"
                    )
    nc.finalize()
    return nc


def get_nc() -> bass.Bass:
    if "nc" not in _CACHE:
        _CACHE["nc"] = _build_nc()
    return _CACHE["nc"]


def _host_prep(weight, gamma, beta, running_mean, running_var):
    """Binarize standardized weights, fold sw + BN into scale/bias."""
    wf = weight.reshape(COUT, -1).astype(np.float64)
    n = wf.shape[1]
    mean = wf.mean(axis=1, keepdims=True)
    d = wf - mean
    sgn = np.where(d >= 0, 1.0, -1.0)
    std = np.sqrt((d * d).sum(axis=1, keepdims=True) / (n - 1))
    bw = d / std
    sw = np.exp2(np.round(np.log2(np.abs(bw).mean(axis=1))))  # [COUT]
    inv = gamma.astype(np.float64) / np.sqrt(running_var.astype(np.float64) + BN_EPS)
    scale = (sw * inv).astype(np.float32)
    bias = (beta.astype(np.float64) - running_mean.astype(np.float64) * inv).astype(
        np.float32
    )

    # wts[p, (co, k, j, m)] = sgn[co*128+m, (j*128+p)*9 + k]
    w6 = sgn.reshape(NCO, 128, NCI, 128, KTAPS)  # [co, m, j, p, k]
    wts = (
        np.ascontiguousarray(np.transpose(w6, (3, 0, 4, 2, 1)))  # p co k j m
        .reshape(128, KTAPS * NCO * NCI * 128)
        .astype(FP8NP)
    )
    # sb[m, co] = scale chunk, sb[m, NCO+co] = bias chunk
    sbarr = np.concatenate(
        [scale.reshape(NCO, 128).T, bias.reshape(NCO, 128).T], axis=1
    ).astype(np.float32)
    sbarr = np.ascontiguousarray(sbarr)
    return wts, sbarr


def _host_signs(x):
    """fp8 +-1 sign planes, zero-padded 58x57 rows, cin-chunk interleaved.

    out[b, p, t, j] = fp8(sign(x[b, j*128+p, r-1, c-1])) at t = GUARD+r*57+c
    for the interior, 0 elsewhere (pads/guards), matching torch.sign
    (sign(0) = 0).
    """
    xv = x.reshape(B, NCI, 128, H, W)
    xs = ((xv < 0).astype(np.uint8) * 0x80) | ((xv != 0).astype(np.uint8) * 0x38)
    out = np.zeros((B, 128, XT, NCI), np.uint8)
    interior = out[:, :, GUARD : GUARD + IMG, :].reshape(B, 128, HP, RW, NCI)
    interior[:, :, 1 : H + 1, 1 : W + 1, :] = xs.transpose(0, 2, 3, 4, 1)
    return out.reshape(B, 128, XT * NCI).view(FP8NP)


def run(x, weight, gamma, beta, running_mean, running_var, trace=False, **tkw):
    x = np.asarray(x, dtype=np.float32)
    wts, sbarr = _host_prep(
        np.asarray(weight, dtype=np.float32),
        np.asarray(gamma, dtype=np.float32),
        np.asarray(beta, dtype=np.float32),
        np.asarray(running_mean, dtype=np.float32),
        np.asarray(running_var, dtype=np.float32),
    )
    x8 = _host_signs(x)
    in_maps = [
        {
            "xin8": x8[c * BPC : (c + 1) * BPC],
            "wts": wts,
            "sb": sbarr,
        }
        for c in range(NCORES)
    ]
    nc = get_nc()
    res = run_bass_kernel_spmd(nc, in_maps, list(range(NCORES)), trace=trace, **tkw)
    y = np.concatenate([r["yout"] for r in res.results], axis=0)
    return y.astype(np.float32, copy=False), res


def kernel(x, weight, gamma, beta, running_mean, running_var):
    y, _ = run(x, weight, gamma, beta, running_mean, running_var)
    return y
